# revision 17
# baseline (speedup 1.0000x reference)
"""GraphSAGE 2-block GNN (nn_BaselineModel_80607946211554) on 8 TRN2 NeuronCores.

Strategy: destination-node sharding. Each core owns a contiguous range of
6250 nodes. Node-feature tables (x, and intermediate h tables) are replicated
in each core's DRAM in a "slab" layout: node n -> table row (n//6250)*6272 +
n%6250, with 22 zero pad rows per slab. Neighbor aggregation is done by
dma_gather of source rows (edges sorted by destination, host-preprocessed)
followed by a fixed ones-block matmul (sums blocks of 4 slots, col-tiled on
the PE array) and a per-window indicator matmul mapping blocks to
destinations. SAGE linear layers run feature-major (weights stationary on the
PE). Intermediate node tables are rebuilt across cores with AllGather
collectives. Graph pooling is a one-hot matmul; the tiny MLP head + softmax is
computed redundantly on every core.

v2: bf16 end-to-end (tables, gathers, matmuls; fp32 PSUM accumulation and
fp32 head), whole-conv gather-index preload, pre-wrapped per-window indicator
and pooling tables (single DMA issue each), full-section gather calls.

Self-contained: hardcodes all shapes for the fixed problem instance.
"""
import os
import sys
import types
import numpy as np
import ml_dtypes

BF = ml_dtypes.bfloat16

N = 50000
E = 1600000
G = 256
F = 128
HID = 128
C = 10
NCORES = 8
NPC = N // NCORES            # 6250 nodes per core
SLAB = 6272                  # slab rows (6250 + 22 zero pad)
NT = NCORES * SLAB           # 50176 table rows
LO = 4 * SLAB                # 25088; table rows < LO hold nodes < 25000
PADROW = 6250                # zero row (local index in both lo/hi views)
P = 128
NW = (NPC + P - 1) // P      # 49 dst windows per core
EPS = 1e-5

_prog_cache = {}


# ----------------------------------------------------------------- host prep
def _wrap_idx(sec):
    """int64 slot values (len mult of 16) -> [128, n/16] int16 wrapped layout."""
    n = len(sec)
    arr = sec.reshape(n // 16, 16).T.astype(np.int16)   # [16, n/16]
    return np.tile(arr, (8, 1))                          # [128, n/16]


def _build_schedule(src, dst, invd_full):
    """Shared static schedule + per-core gather/indicator data.
    ind2 entries carry 1/deg(dst) so stage-2 emits the neighbourhood mean."""
    core_edges = []
    SL = np.zeros((NCORES, NW), np.int64)
    SH = np.zeros((NCORES, NW), np.int64)
    for c in range(NCORES):
        m = (dst >= c * NPC) & (dst < (c + 1) * NPC)
        s = src[m].astype(np.int64)
        d = (dst[m] - c * NPC).astype(np.int64)
        hi = (s >= N // 2).astype(np.int64)
        w = d >> 7
        order = np.lexsort((d, hi, w))
        s, d, hi, w = s[order], d[order], hi[order], w[order]
        core_edges.append((s, d, hi, w))
        cnt = np.bincount(d * 2 + hi, minlength=NPC * 2).reshape(NPC, 2)
        pl = ((cnt + 3) >> 2) << 2
        plp = np.zeros((NW * P, 2), np.int64)
        plp[:NPC] = pl
        plw = plp.reshape(NW, P, 2).sum(1)
        SL[c], SH[c] = plw[:, 0], plw[:, 1]

    nL = np.maximum(((SL.max(0) + 127) // P) * P, P).astype(np.int64)
    nH = np.maximum(((SH.max(0) + 127) // P) * P, P).astype(np.int64)
    S = nL + nH
    B = S // 4
    T = (B + 127) // P
    colL = np.zeros(NW, np.int64)
    colH = np.zeros(NW, np.int64)
    off = 0
    for w in range(NW):
        colL[w] = off
        off += nL[w] // 16
        colH[w] = off
        off += nH[w] // 16
    idx_cols = off
    i2off = np.zeros(NW, np.int64)
    o = 0
    for w in range(NW):
        i2off[w] = o
        o += T[w] * P
    i2rows = o

    sched = dict(nL=nL, nH=nH, S=S, B=B, T=T, colL=colL, colH=colH,
                 idx_cols=idx_cols, i2off=i2off, i2rows=i2rows,
                 smax=int(S.max()), tmax=int(T.max()))

    per_core = []
    for c in range(NCORES):
        s, d, hi, w = core_edges[c]
        cnt = np.bincount(d * 2 + hi, minlength=NPC * 2).reshape(NPC, 2)
        pl = ((cnt + 3) >> 2) << 2
        plp = np.zeros((NW * P, 2), np.int64)
        plp[:NPC] = pl
        plw3 = plp.reshape(NW, P, 2)
        gstart = np.cumsum(plw3, axis=1) - plw3           # [NW,128,2]
        key = d * 2 + hi
        if len(key):
            grp_change = np.r_[True, key[1:] != key[:-1]]
            gidx = np.cumsum(grp_change) - 1
            first_pos = np.flatnonzero(grp_change)
            rank = np.arange(len(d)) - first_pos[gidx]
        else:
            rank = np.zeros(0, np.int64)
        pos = gstart[w, d & 127, hi] + rank
        trow = (s // NPC) * SLAB + s % NPC
        val = np.where(hi == 1, trow - LO, trow)

        idx_arr = np.zeros((P, idx_cols), np.int16)
        ind2 = np.zeros((i2rows, P), np.float32)
        for wi in range(NW):
            mw = w == wi
            mL = mw & (hi == 0)
            mH = mw & (hi == 1)
            secL = np.full(nL[wi], PADROW, np.int64)
            secH = np.full(nH[wi], PADROW, np.int64)
            secL[pos[mL]] = val[mL]
            secH[pos[mH]] = val[mH]
            idx_arr[:, colL[wi]:colL[wi] + nL[wi] // 16] = _wrap_idx(secL)
            idx_arr[:, colH[wi]:colH[wi] + nH[wi] // 16] = _wrap_idx(secH)
            d0, d1 = wi * P, min((wi + 1) * P, NPC)
            dloc = np.arange(d1 - d0)
            bL = np.repeat(dloc, pl[d0:d1, 0] // 4)
            bH = np.repeat(dloc, pl[d0:d1, 1] // 4)
            b2d = np.full(T[wi] * P, -1, np.int64)
            b2d[:len(bL)] = bL
            b2d[nL[wi] // 4:nL[wi] // 4 + len(bH)] = bH
            rows = np.arange(T[wi] * P)
            vmask = b2d >= 0
            blk = ind2[i2off[wi]:i2off[wi] + T[wi] * P]
            blk[rows[vmask], b2d[vmask]] = invd_full[c * NPC + wi * P + b2d[vmask]]
        # pre-wrap indicator to partition-major: ind2w[p, j*128+q] = ind2[j*128+p, q]
        ind2w = (ind2.reshape(i2rows // P, P, P)
                 .transpose(1, 0, 2).reshape(P, i2rows).astype(BF))
        per_core.append(dict(idx=idx_arr, ind2w=ind2w))
    return sched, per_core


def _host_inputs(inputs):
    f32 = lambda a: np.asarray(a, np.float32)
    bf16 = lambda a: np.asarray(a, np.float32).astype(BF)
    x = f32(inputs["x"])
    ei = np.asarray(inputs["edge_index"], np.int64)
    batch = np.asarray(inputs["batch"], np.int64)
    src, dst = ei[0], ei[1]

    deg = np.bincount(dst, minlength=N).astype(np.float32)
    invd_full = (1.0 / np.maximum(deg, 1.0)).astype(np.float32)

    sched, per_core = _build_schedule(src, dst, invd_full)

    xt = np.zeros((NT, F), BF)
    for r in range(NCORES):
        xt[r * SLAB:r * SLAB + NPC] = x[r * NPC:(r + 1) * NPC].astype(BF)

    o4 = np.zeros((P, 32), np.float32)
    for e in range(P):
        o4[e, e // 4] = 1.0
    ident = np.eye(P, dtype=np.float32)

    # BN folding
    s_bn = f32(inputs["bn_gamma"]) / np.sqrt(f32(inputs["bn_rv"]) + EPS)
    t_bn = f32(inputs["bn_beta"]) - f32(inputs["bn_rm"]) * s_bn
    bns2 = s_bn.reshape(2, P).T.copy()     # [128, 2]
    bnt2 = t_bn.reshape(2, P).T.copy()

    shared = {
        "xt": xt, "o4": o4.astype(BF), "ident": ident.astype(BF),
        "identf": ident, "bns2": bns2, "bnt2": bnt2,
        "l1w": f32(inputs["lin1_W"]), "l1b": f32(inputs["lin1_b"]),
        "l2w": f32(inputs["lin2_W"]), "l2b": f32(inputs["lin2_b"]),
    }
    for b in (0, 1):
        for nm in ("Wl1", "Wr1", "Wl2", "Wr2", "Wlin"):
            shared[f"b{b}_{nm}"] = bf16(inputs[f"b{b}_{nm}"])
        for nm in ("b1", "b2", "blin"):
            shared[f"b{b}_{nm}"] = f32(inputs[f"b{b}_{nm}"])

    in_maps = []
    for c in range(NCORES):
        xoT = np.zeros((F, SLAB), BF)
        xoT[:, :NPC] = x[c * NPC:(c + 1) * NPC].T.astype(BF)
        pool_ind = np.zeros((NW, P, G), np.float32)
        bt = batch[c * NPC:(c + 1) * NPC]
        btp = np.full(NW * P, -1, np.int64)
        btp[:NPC] = bt
        btp2 = btp.reshape(NW, P)
        for wi in range(NW):
            vm = btp2[wi] >= 0
            pool_ind[wi, np.arange(P)[vm], btp2[wi][vm]] = 1.0
        # pre-wrap pooling indicator: pindw[p, w*G+g] = pool_ind[w, p, g]
        pindw = pool_ind.transpose(1, 0, 2).reshape(P, NW * G).astype(BF)
        im = dict(shared)
        im.update({
            "xoT": xoT, "pindw": pindw,
            "idx": per_core[c]["idx"], "ind2w": per_core[c]["ind2w"],
        })
        in_maps.append(im)
    return sched, in_maps


# ------------------------------------------------------------- bass program
def _build_program(sched, n_convs=4, debug_tables=False):
    import concourse.bass as bass
    import concourse.mybir as mybir
    import concourse.tile as tile
    from concourse import bacc
    from concourse import library_config
    from contextlib import ExitStack

    dt = mybir.dt
    DT = dt.float32
    BT = dt.bfloat16
    Alu = mybir.AluOpType

    nL, nH, S, B, T = (sched[k] for k in ("nL", "nH", "S", "B", "T"))
    colL, colH, i2off = sched["colL"], sched["colH"], sched["i2off"]
    SMAX = sched["smax"]
    TMAX = sched["tmax"]
    IDXC = int(sched["idx_cols"])
    I2R = int(sched["i2rows"])

    nc = bacc.Bacc("TRN2", debug=False, num_swdge_queues=4,
                   dynamic_dma_scratch_size=65536)

    # ---- parameters
    xt = nc.declare_dram_parameter("xt", [NT, F], BT, isOutput=False)
    xoT = nc.declare_dram_parameter("xoT", [F, SLAB], BT, isOutput=False)
    idxp = nc.declare_dram_parameter("idx", [P, IDXC], dt.int16, isOutput=False)
    ind2p = nc.declare_dram_parameter("ind2w", [P, I2R], BT, isOutput=False)
    pindp = nc.declare_dram_parameter("pindw", [P, NW * G], BT, isOutput=False)
    o4p = nc.declare_dram_parameter("o4", [P, 32], BT, isOutput=False)
    identp = nc.declare_dram_parameter("ident", [P, P], BT, isOutput=False)
    identfp = nc.declare_dram_parameter("identf", [P, P], DT, isOutput=False)
    wp = {}
    for b in (0, 1):
        for nm, shp, ty in (("Wl1", [F, HID], BT), ("Wr1", [F, HID], BT),
                            ("b1", [HID], DT),
                            ("Wl2", [HID, HID], BT), ("Wr2", [HID, HID], BT),
                            ("b2", [HID], DT),
                            ("Wlin", [2 * HID, HID], BT), ("blin", [HID], DT)):
            wp[f"b{b}_{nm}"] = nc.declare_dram_parameter(f"b{b}_{nm}", shp, ty, isOutput=False)
    bns2p = nc.declare_dram_parameter("bns2", [P, 2], DT, isOutput=False)
    bnt2p = nc.declare_dram_parameter("bnt2", [P, 2], DT, isOutput=False)
    l1wp = nc.declare_dram_parameter("l1w", [2 * HID, HID], DT, isOutput=False)
    l1bp = nc.declare_dram_parameter("l1b", [HID], DT, isOutput=False)
    l2wp = nc.declare_dram_parameter("l2w", [HID, C], DT, isOutput=False)
    l2bp = nc.declare_dram_parameter("l2b", [C], DT, isOutput=False)

    out = nc.declare_dram_parameter("out", [G, C], DT, isOutput=True)
    if debug_tables:
        dbgA = nc.declare_dram_parameter("dbgA", [NT, F], BT, isOutput=True)
        dbgB = nc.declare_dram_parameter("dbgB", [NT, F], BT, isOutput=True)

    with tile.TileContext(nc) as tc, ExitStack() as ctx:
        sb = ctx.enter_context(tc.tile_pool(name="sb", bufs=1))
        sb_feat = ctx.enter_context(tc.tile_pool(name="sb_feat", bufs=1))
        sb_g = ctx.enter_context(tc.tile_pool(name="sb_g", bufs=3))
        sb_i2 = ctx.enter_context(tc.tile_pool(name="sb_i2", bufs=4))
        sb_bs = ctx.enter_context(tc.tile_pool(name="sb_bs", bufs=4))
        sb_ms = ctx.enter_context(tc.tile_pool(name="sb_ms", bufs=3))
        ps_bs = ctx.enter_context(tc.tile_pool(name="ps_bs", bufs=3, space="PSUM"))
        ps_agg = ctx.enter_context(tc.tile_pool(name="ps_agg", bufs=2, space="PSUM"))
        ps_mm = ctx.enter_context(tc.tile_pool(name="ps_mm", bufs=2, space="PSUM"))
        ps_pool = ctx.enter_context(tc.tile_pool(name="ps_pool", bufs=1, space="PSUM"))
        dram = ctx.enter_context(tc.tile_pool(name="dram", bufs=1, space="DRAM"))

        nc.gpsimd.load_library(library_config.mlp)

        # ---- constants into SBUF
        o4_t = sb.tile([P, 32], BT)
        nc.sync.dma_start(o4_t[:], o4p[:])
        id_t = sb.tile([P, P], BT)
        nc.sync.dma_start(id_t[:], identp[:])
        idf_t = sb.tile([P, P], DT)
        nc.sync.dma_start(idf_t[:], identfp[:])
        ix_t = sb.tile([P, IDXC], dt.int16)
        nc.sync.dma_start(ix_t[:], idxp[:])
        pind_t = sb.tile([P, NW * G], BT)
        nc.sync.dma_start(pind_t[:], pindp[:])
        wt = {}
        for b in (0, 1):
            for nm in ("Wl1", "Wr1", "Wl2", "Wr2"):
                w_t = sb.tile([P, P], BT, name=f"w{b}{nm}")
                nc.sync.dma_start(w_t[:], wp[f"b{b}_{nm}"][:])
                wt[f"b{b}_{nm}"] = w_t
            wlin_t = sb.tile([P, 2, P], BT, name=f"w{b}lin")
            nc.sync.dma_start(wlin_t[:, 0, :], wp[f"b{b}_Wlin"][0:P, :])
            nc.sync.dma_start(wlin_t[:, 1, :], wp[f"b{b}_Wlin"][P:2 * P, :])
            wt[f"b{b}_Wlin"] = wlin_t
            for nm in ("b1", "b2", "blin"):
                b_t = sb.tile([P, 1], DT, name=f"b{b}{nm}")
                nc.sync.dma_start(b_t[:], wp[f"b{b}_{nm}"][:, None])
                wt[f"b{b}_{nm}"] = b_t
        bns_t = sb.tile([P, 2], DT)
        nc.sync.dma_start(bns_t[:], bns2p[:])
        bnt_t = sb.tile([P, 2], DT)
        nc.sync.dma_start(bnt_t[:], bnt2p[:])
        l1w_t = sb.tile([P, 2, P], DT)
        nc.sync.dma_start(l1w_t[:, 0, :], l1wp[0:P, :])
        nc.sync.dma_start(l1w_t[:, 1, :], l1wp[P:2 * P, :])
        l1b_t = sb.tile([P, 1], DT)
        nc.sync.dma_start(l1b_t[:], l1bp[:, None])
        l2w_t = sb.tile([P, C], DT)
        nc.sync.dma_start(l2w_t[:], l2wp[:])
        l2b_t = sb.tile([P, 1], DT)
        nc.sync.dma_start(l2b_t[0:C, :], l2bp[:, None])

        # feature-major activation buffers [128, SLAB]
        featA = sb_feat.tile([P, SLAB], BT)   # x_ownT, later h (block0 out), h' ...
        featB = sb_feat.tile([P, SLAB], BT)   # h1, h1'
        featC = sb_feat.tile([P, SLAB], BT)   # h2, h2'
        nc.sync.dma_start(featA[:], xoT[:])

        zero_t = sb.tile([P, P], BT)
        nc.vector.memset(zero_t[:], 0.0)

        # DRAM scratch
        cA = dram.tile([SLAB, F], BT)
        cB = dram.tile([SLAB, F], BT)
        tabA = dram.tile([NT, F], BT, addr_space="Shared")
        tabB = dram.tile([NT, F], BT, addr_space="Shared")
        tabC = dram.tile([NT, F], BT, addr_space="Shared")
        pc_in = dram.tile([P, 2 * G], DT)
        pc_out = dram.tile([P, 2 * G], DT, addr_space="Shared")
        nc.sync.dma_start(cA[NPC:SLAB, :], zero_t[0:SLAB - NPC, :])
        nc.sync.dma_start(cB[NPC:SLAB, :], zero_t[0:SLAB - NPC, :])

        qctr = [0]
        ACT_COPY = mybir.ActivationFunctionType.Copy

        def jk_window(w, h1, h2, hout, Wlin, bcol, contrib, pool_ps):
            h_ps = ps_mm.tile([P, P], dt.float32, name="jk_ps", tag="mm")
            nc.tensor.matmul(h_ps[:], Wlin[:, 0, :], h1[:, w * P:(w + 1) * P], start=True, stop=False)
            nc.tensor.matmul(h_ps[:], Wlin[:, 1, :], h2[:, w * P:(w + 1) * P], start=False, stop=True)
            nc.vector.tensor_scalar(hout[:, w * P:(w + 1) * P], h_ps[:], bcol[:], 0.0, Alu.add, Alu.max)
            hnm_ps = ps_mm.tile([P, P], BT, name="jknm_ps", tag="mm")
            nc.tensor.transpose(hnm_ps[:], hout[:, w * P:(w + 1) * P], id_t[:])
            hnm_sb = sb_ms.tile([P, P], BT, name="jknm_sb")
            nc.vector.tensor_copy(hnm_sb[:], hnm_ps[:])
            if contrib is not None:
                rows = min(P, NPC - w * P)
                nc.scalar.dma_start(contrib[w * P:w * P + rows, :], hnm_sb[0:rows, :])
            nc.tensor.matmul(pool_ps[:], hnm_sb[:], pind_t[:, w * G:(w + 1) * G],
                             start=(w == 0), stop=(w == NW - 1))

        def conv(tab, in_feat, out_feat, Wl, Wr, bcol, contrib, jk_spec=None):
            """One SAGE conv: out_feat[:, n] = relu(mean@Wl + in@Wr + b).
            If contrib is not None also write node-major rows there.
            If jk_spec is given, the JumpingKnowledge/pool window of the
            previous conv pair is interleaved after each window."""
            if jk_spec is not None:
                pool_ps = ps_pool.tile([P, G], dt.float32, name="pool_ps")
            for w in range(NW):
                nLw, nHw, Sw, Tw = int(nL[w]), int(nH[w]), int(S[w]), int(T[w])
                g_t = sb_g.tile([P, SMAX // P, P], BT, name="g_t")
                for nsec, col0, slot0, view in (
                        (nLw, int(colL[w]), 0, tab[0:LO]),
                        (nHw, int(colH[w]), nLw // P, tab[LO:NT])):
                    # two chunks per section: smaller SWDGE programs give the
                    # ring lookahead room so drain overlaps compute
                    ch0 = ((nsec // 2 + 127) // P) * P
                    for off, ln in ((0, min(ch0, nsec)), (ch0, nsec - ch0)):
                        if ln <= 0:
                            continue
                        nc.gpsimd.dma_gather(
                            g_t[:, slot0 + off // P:slot0 + (off + ln) // P, :], view,
                            ix_t[:, col0 + off // 16:col0 + (off + ln) // 16],
                            ln, ln, P, single_packet=False,
                            queue_num=qctr[0] % 4)
                        qctr[0] += 1

                i2 = sb_i2.tile([P, TMAX * P], BT, name="i2")
                r0 = int(i2off[w])
                nc.sync.dma_start(i2[:, 0:Tw * P], ind2p[:, r0:r0 + Tw * P])

                agg = ps_agg.tile([P, P], dt.float32, name="agg")
                ngrp = Sw // P
                for t in range(Tw):
                    jn = min(4, ngrp - t * 4)
                    bs_ps = ps_bs.tile([P, P], dt.float32, name="bs_ps")
                    for jj in range(jn):
                        j = t * 4 + jj
                        nc.tensor.matmul(
                            bs_ps[32 * jj:32 * jj + 32, :], o4_t[:], g_t[:, j, :],
                            start=True, stop=True, tile_position=(0, 32 * jj))
                    Kt = jn * 32
                    bs_sb = sb_bs.tile([P, P], BT, name="bs_sb")
                    nc.scalar.activation(bs_sb[0:Kt, :], bs_ps[0:Kt, :], ACT_COPY)
                    nc.tensor.matmul(agg[:], bs_sb[0:Kt, :], i2[0:Kt, t * P:t * P + P],
                                     start=(t == 0), stop=(t == Tw - 1))

                mT_sb = sb_ms.tile([P, P], BT, name="mT_sb")
                nc.vector.tensor_copy(mT_sb[:], agg[:])
                h_ps = ps_mm.tile([P, P], dt.float32, name="h_ps", tag="mm")
                nc.tensor.matmul(h_ps[:], Wl[:], mT_sb[:], start=True, stop=False)
                nc.tensor.matmul(h_ps[:], Wr[:], in_feat[:, w * P:(w + 1) * P], start=False, stop=True)
                nc.vector.tensor_scalar(out_feat[:, w * P:(w + 1) * P], h_ps[:], bcol[:], 0.0, Alu.add, Alu.max)
                if contrib is not None:
                    rows = min(P, NPC - w * P)
                    hnm_ps = ps_mm.tile([P, P], BT, name="hnm_ps", tag="mm")
                    nc.tensor.transpose(hnm_ps[:], out_feat[:, w * P:(w + 1) * P], id_t[:])
                    hnm_sb = sb_ms.tile([P, P], BT, name="hnm_sb")
                    nc.scalar.activation(hnm_sb[:], hnm_ps[:], ACT_COPY)
                    nc.scalar.dma_start(contrib[w * P:w * P + rows, :], hnm_sb[0:rows, :])
                if jk_spec is not None:
                    jk_window(w, *jk_spec[:6], pool_ps)
            if jk_spec is not None:
                nc.vector.tensor_copy(jk_spec[6][:], pool_ps[:])

        def allgather(contrib, tab):
            nc.gpsimd.collective_compute(
                "AllGather", Alu.bypass, ins=[contrib[:]], outs=[tab[:]],
                replica_groups=[list(range(NCORES))])

        # ---------------- block 0
        conv(xt, featA, featB, wt["b0_Wl1"], wt["b0_Wr1"], wt["b0_b1"], cA)   # h1
        allgather(cA, tabA)
        if n_convs >= 2:
            p0_sb = sb.tile([P, G], DT)
            # conv2 (h2) with JK/pool of block0 interleaved per window
            conv(tabA, featB, featC, wt["b0_Wl2"], wt["b0_Wr2"], wt["b0_b2"], None,
                 jk_spec=(featB, featC, featA, wt["b0_Wlin"], wt["b0_blin"], cB, p0_sb))
            allgather(cB, tabB)
        if n_convs >= 3:
            conv(tabB, featA, featB, wt["b1_Wl1"], wt["b1_Wr1"], wt["b1_b1"], cA)  # h1'
            allgather(cA, tabC)
        if n_convs >= 4:
            p1_sb = sb.tile([P, G], DT)
            conv(tabC, featB, featC, wt["b1_Wl2"], wt["b1_Wr2"], wt["b1_b2"], None,
                 jk_spec=(featB, featC, featA, wt["b1_Wlin"], wt["b1_blin"], None, p1_sb))

            # ---------------- pooling allreduce + head
            nc.sync.dma_start(pc_in[:, 0:G], p0_sb[:])
            nc.sync.dma_start(pc_in[:, G:2 * G], p1_sb[:])
            nc.gpsimd.collective_compute(
                "AllReduce", Alu.add, ins=[pc_in[:]], outs=[pc_out[:]],
                replica_groups=[list(range(NCORES))])
            pools_sb = sb.tile([P, 2 * G], DT)
            nc.sync.dma_start(pools_sb[:], pc_out[:])

            # BN (folded) per feature chunk
            gbn = sb.tile([P, 2, G], DT)
            for k in range(2):
                nc.vector.tensor_scalar(gbn[:, k, :], pools_sb[:, k * G:(k + 1) * G],
                                        bns_t[:, k:k + 1], bnt_t[:, k:k + 1],
                                        Alu.mult, Alu.add)
            l1_ps = ps_mm.tile([P, G], dt.float32, name="l1_ps", tag="mm")
            for k in range(2):
                nc.tensor.matmul(l1_ps[:], l1w_t[:, k, :], gbn[:, k, :],
                                 start=(k == 0), stop=(k == 1))
            z1 = sb.tile([P, G], DT)
            nc.vector.tensor_scalar(z1[:], l1_ps[:], l1b_t[:], 0.0, Alu.add, Alu.max)
            l2_ps = ps_mm.tile([P, G], dt.float32, name="l2_ps", tag="mm")
            nc.tensor.matmul(l2_ps[0:C, :], l2w_t[:], z1[:], start=True, stop=True)
            z2 = sb.tile([P, G], DT)
            nc.vector.tensor_scalar(z2[0:C, :], l2_ps[0:C, :], l2b_t[0:C, :], None, Alu.add)

            # softmax over C (partition dim) -> transpose to [G, C] first
            for half in range(2):
                zt_ps = ps_mm.tile([P, C], dt.float32, name="zt_ps", tag="mm")
                nc.tensor.transpose(zt_ps[:, 0:C], z2[0:C, half * P:(half + 1) * P], idf_t[0:C, 0:C])
                znm = sb.tile([P, C], DT, name=f"znm{half}")
                nc.vector.tensor_copy(znm[:], zt_ps[:, 0:C])
                nmax = sb.tile([P, 1], DT, name=f"nmax{half}")
                nc.vector.tensor_reduce(nmax[:], znm[:], mybir.AxisListType.X, Alu.max, negate=True)
                e_t = sb.tile([P, C], DT, name=f"e_t{half}")
                nc.scalar.activation(e_t[:], znm[:], mybir.ActivationFunctionType.Exp,
                                     bias=nmax[:], scale=1.0)
                ssum = sb.tile([P, 1], DT, name=f"ssum{half}")
                nc.vector.tensor_reduce(ssum[:], e_t[:], mybir.AxisListType.X, Alu.add)
                rcp = sb.tile([P, 1], DT, name=f"rcp{half}")
                nc.vector.reciprocal(rcp[:], ssum[:])
                sm = sb.tile([P, C], DT, name=f"sm{half}")
                nc.vector.tensor_scalar(sm[:], e_t[:], rcp[:], None, Alu.mult)
                nc.sync.dma_start(out[half * P:(half + 1) * P, :], sm[:])

        if debug_tables:
            nc.sync.dma_start(dbgA[:], tabA[:])
            if n_convs >= 2:
                nc.sync.dma_start(dbgB[:], tabB[:])

    nc.compile()
    return nc


# ------------------------------------------------------------------ runtime
def _install_profile_hook():
    try:
        from trn_agent_boot.trn_boot import _ntff_profile_via_ctypes
        hook = _ntff_profile_via_ctypes("/opt/axon/libaxon_pjrt.so")
        m = types.ModuleType("antenv.axon_hooks")
        m.get_axon_ntff_profile_hook = lambda: hook
        sys.modules.setdefault("antenv.axon_hooks", m)
    except Exception:
        pass


def kernel(**inputs):
    from concourse.bass_utils import run_bass_kernel_spmd

    n_convs = int(os.environ.get("KNC_CONVS", "4"))
    debug_tables = bool(int(os.environ.get("KDBG", "0")))
    trace = bool(int(os.environ.get("KTRACE", "0")))
    if trace:
        _install_profile_hook()

    sched, in_maps = _host_inputs(inputs)

    key = (n_convs, debug_tables,
           tuple(int(v) for v in sched["S"][:8]), int(sched["i2rows"]))
    nc = _prog_cache.get(key)
    if nc is None:
        nc = _build_program(sched, n_convs=n_convs, debug_tables=debug_tables)
        _prog_cache[key] = nc

    res = run_bass_kernel_spmd(nc, in_maps, list(range(NCORES)), trace=trace)
    kernel.last_result = res
    out = res.results[0]["out"].astype(np.float32)
    return out


# revision 25
# speedup vs baseline: 1.0240x; 1.0240x over previous
"""GraphSAGE 2-block GNN (nn_BaselineModel_80607946211554) on 8 TRN2 NeuronCores.

Strategy: destination-node sharding. Each core owns a contiguous range of
6250 nodes. Node-feature tables (x, and intermediate h tables) are replicated
in each core's DRAM in a "slab" layout: node n -> table row (n//6250)*6272 +
n%6250, with 22 zero pad rows per slab. Neighbor aggregation is done by
dma_gather of source rows (edges sorted by destination, host-preprocessed)
followed by a fixed ones-block matmul (sums blocks of 4 slots, col-tiled on
the PE array) and a per-window indicator matmul mapping blocks to
destinations. SAGE linear layers run feature-major (weights stationary on the
PE). Intermediate node tables are rebuilt across cores with AllGather
collectives. Graph pooling is a one-hot matmul; the tiny MLP head + softmax is
computed redundantly on every core.

v2: bf16 end-to-end (tables, gathers, matmuls; fp32 PSUM accumulation and
fp32 head), whole-conv gather-index preload, pre-wrapped per-window indicator
and pooling tables (single DMA issue each), full-section gather calls.

Self-contained: hardcodes all shapes for the fixed problem instance.
"""
import os
import sys
import types
import numpy as np
import ml_dtypes

BF = ml_dtypes.bfloat16

N = 50000
E = 1600000
G = 256
F = 128
HID = 128
C = 10
NCORES = 8
NPC = N // NCORES            # 6250 nodes per core
SLAB = 6272                  # slab rows (6250 + 22 zero pad)
NT = NCORES * SLAB           # 50176 table rows
LO = 4 * SLAB                # 25088; table rows < LO hold nodes < 25000
PADROW = 6250                # zero row (local index in both lo/hi views)
P = 128
NW = (NPC + P - 1) // P      # 49 dst windows per core
EPS = 1e-5

_prog_cache = {}


# ----------------------------------------------------------------- host prep
def _wrap_idx(sec):
    """int64 slot values (len mult of 16) -> [128, n/16] int16 wrapped layout."""
    n = len(sec)
    arr = sec.reshape(n // 16, 16).T.astype(np.int16)   # [16, n/16]
    return np.tile(arr, (8, 1))                          # [128, n/16]


def _build_schedule(src, dst, invd_full):
    """Shared static schedule + per-core gather/indicator data.
    ind2 entries carry 1/deg(dst) so stage-2 emits the neighbourhood mean."""
    core_edges = []
    SL = np.zeros((NCORES, NW), np.int64)
    SH = np.zeros((NCORES, NW), np.int64)
    for c in range(NCORES):
        m = (dst >= c * NPC) & (dst < (c + 1) * NPC)
        s = src[m].astype(np.int64)
        d = (dst[m] - c * NPC).astype(np.int64)
        hi = (s >= N // 2).astype(np.int64)
        w = d >> 7
        order = np.lexsort((d, hi, w))
        s, d, hi, w = s[order], d[order], hi[order], w[order]
        core_edges.append((s, d, hi, w))
        cnt = np.bincount(d * 2 + hi, minlength=NPC * 2).reshape(NPC, 2)
        pl = ((cnt + 3) >> 2) << 2
        plp = np.zeros((NW * P, 2), np.int64)
        plp[:NPC] = pl
        plw = plp.reshape(NW, P, 2).sum(1)
        SL[c], SH[c] = plw[:, 0], plw[:, 1]

    nL = np.maximum(((SL.max(0) + 127) // P) * P, P).astype(np.int64)
    nH = np.maximum(((SH.max(0) + 127) // P) * P, P).astype(np.int64)
    S = nL + nH
    B = S // 4
    T = (B + 127) // P
    colL = np.zeros(NW, np.int64)
    colH = np.zeros(NW, np.int64)
    off = 0
    for w in range(NW):
        colL[w] = off
        off += nL[w] // 16
        colH[w] = off
        off += nH[w] // 16
    idx_cols = off
    i2off = np.zeros(NW, np.int64)
    o = 0
    for w in range(NW):
        i2off[w] = o
        o += T[w] * P
    i2rows = o

    sched = dict(nL=nL, nH=nH, S=S, B=B, T=T, colL=colL, colH=colH,
                 idx_cols=idx_cols, i2off=i2off, i2rows=i2rows,
                 smax=int(S.max()), tmax=int(T.max()))

    per_core = []
    for c in range(NCORES):
        s, d, hi, w = core_edges[c]
        cnt = np.bincount(d * 2 + hi, minlength=NPC * 2).reshape(NPC, 2)
        pl = ((cnt + 3) >> 2) << 2
        plp = np.zeros((NW * P, 2), np.int64)
        plp[:NPC] = pl
        plw3 = plp.reshape(NW, P, 2)
        gstart = np.cumsum(plw3, axis=1) - plw3           # [NW,128,2]
        key = d * 2 + hi
        if len(key):
            grp_change = np.r_[True, key[1:] != key[:-1]]
            gidx = np.cumsum(grp_change) - 1
            first_pos = np.flatnonzero(grp_change)
            rank = np.arange(len(d)) - first_pos[gidx]
        else:
            rank = np.zeros(0, np.int64)
        pos = gstart[w, d & 127, hi] + rank
        trow = (s // NPC) * SLAB + s % NPC
        val = np.where(hi == 1, trow - LO, trow)

        idx_arr = np.zeros((P, idx_cols), np.int16)
        ind2 = np.zeros((i2rows, P), np.float32)
        for wi in range(NW):
            mw = w == wi
            mL = mw & (hi == 0)
            mH = mw & (hi == 1)
            d0, d1w = wi * P, min((wi + 1) * P, NPC)
            own_l = int(pl[d0:d1w, 0].sum())
            own_h = int(pl[d0:d1w, 1].sum())
            # tail slots beyond this core's own (group-padded) usage are -1:
            # the gather ucode truncates trailing negatives before descriptor
            # generation, so they cost no SWDGE emission time. The stale g_t
            # data they leave is nullified by this core's zero ind2 entries.
            KTRUNC = int(os.environ.get("KTRUNC", "0"))
            fillv = -1 if KTRUNC else PADROW
            secL = np.full(nL[wi], fillv, np.int64)
            secH = np.full(nH[wi], fillv, np.int64)
            secL[:own_l] = PADROW
            secH[:own_h] = PADROW
            secL[pos[mL]] = val[mL]
            secH[pos[mH]] = val[mH]
            idx_arr[:, colL[wi]:colL[wi] + nL[wi] // 16] = _wrap_idx(secL)
            idx_arr[:, colH[wi]:colH[wi] + nH[wi] // 16] = _wrap_idx(secH)
            d0, d1 = wi * P, min((wi + 1) * P, NPC)
            dloc = np.arange(d1 - d0)
            bL = np.repeat(dloc, pl[d0:d1, 0] // 4)
            bH = np.repeat(dloc, pl[d0:d1, 1] // 4)
            b2d = np.full(T[wi] * P, -1, np.int64)
            b2d[:len(bL)] = bL
            b2d[nL[wi] // 4:nL[wi] // 4 + len(bH)] = bH
            rows = np.arange(T[wi] * P)
            vmask = b2d >= 0
            blk = ind2[i2off[wi]:i2off[wi] + T[wi] * P]
            blk[rows[vmask], b2d[vmask]] = invd_full[c * NPC + wi * P + b2d[vmask]]
        # pre-wrap indicator to partition-major: ind2w[p, j*128+q] = ind2[j*128+p, q]
        ind2w = (ind2.reshape(i2rows // P, P, P)
                 .transpose(1, 0, 2).reshape(P, i2rows).astype(BF))
        per_core.append(dict(idx=idx_arr, ind2w=ind2w))
    return sched, per_core


def _host_inputs(inputs):
    f32 = lambda a: np.asarray(a, np.float32)
    bf16 = lambda a: np.asarray(a, np.float32).astype(BF)
    x = f32(inputs["x"])
    ei = np.asarray(inputs["edge_index"], np.int64)
    batch = np.asarray(inputs["batch"], np.int64)
    src, dst = ei[0], ei[1]

    deg = np.bincount(dst, minlength=N).astype(np.float32)
    invd_full = (1.0 / np.maximum(deg, 1.0)).astype(np.float32)

    sched, per_core = _build_schedule(src, dst, invd_full)

    xt = np.zeros((NT, F), BF)
    for r in range(NCORES):
        xt[r * SLAB:r * SLAB + NPC] = x[r * NPC:(r + 1) * NPC].astype(BF)

    o4 = np.zeros((P, 32), np.float32)
    for e in range(P):
        o4[e, e // 4] = 1.0
    ident = np.eye(P, dtype=np.float32)

    # BN folding
    s_bn = f32(inputs["bn_gamma"]) / np.sqrt(f32(inputs["bn_rv"]) + EPS)
    t_bn = f32(inputs["bn_beta"]) - f32(inputs["bn_rm"]) * s_bn
    bns2 = s_bn.reshape(2, P).T.copy()     # [128, 2]
    bnt2 = t_bn.reshape(2, P).T.copy()

    shared = {
        "xt": xt, "o4": o4.astype(BF), "ident": ident.astype(BF),
        "identf": ident, "bns2": bns2, "bnt2": bnt2,
        "l1w": f32(inputs["lin1_W"]), "l1b": f32(inputs["lin1_b"]),
        "l2w": f32(inputs["lin2_W"]), "l2b": f32(inputs["lin2_b"]),
    }
    for b in (0, 1):
        for nm in ("Wl1", "Wr1", "Wl2", "Wr2", "Wlin"):
            shared[f"b{b}_{nm}"] = bf16(inputs[f"b{b}_{nm}"])
        for nm in ("b1", "b2", "blin"):
            shared[f"b{b}_{nm}"] = f32(inputs[f"b{b}_{nm}"])

    in_maps = []
    for c in range(NCORES):
        xoT = np.zeros((F, SLAB), BF)
        xoT[:, :NPC] = x[c * NPC:(c + 1) * NPC].T.astype(BF)
        pool_ind = np.zeros((NW, P, G), np.float32)
        bt = batch[c * NPC:(c + 1) * NPC]
        btp = np.full(NW * P, -1, np.int64)
        btp[:NPC] = bt
        btp2 = btp.reshape(NW, P)
        for wi in range(NW):
            vm = btp2[wi] >= 0
            pool_ind[wi, np.arange(P)[vm], btp2[wi][vm]] = 1.0
        # pre-wrap pooling indicator: pindw[p, w*G+g] = pool_ind[w, p, g]
        pindw = pool_ind.transpose(1, 0, 2).reshape(P, NW * G).astype(BF)
        im = dict(shared)
        im.update({
            "xoT": xoT, "pindw": pindw,
            "idx": per_core[c]["idx"], "ind2w": per_core[c]["ind2w"],
        })
        in_maps.append(im)
    return sched, in_maps


# ------------------------------------------------------------- bass program
def _build_program(sched, n_convs=4, debug_tables=False):
    import concourse.bass as bass
    import concourse.mybir as mybir
    import concourse.tile as tile
    from concourse import bacc
    from concourse import library_config
    from contextlib import ExitStack

    dt = mybir.dt
    DT = dt.float32
    BT = dt.bfloat16
    Alu = mybir.AluOpType

    nL, nH, S, B, T = (sched[k] for k in ("nL", "nH", "S", "B", "T"))
    colL, colH, i2off = sched["colL"], sched["colH"], sched["i2off"]
    SMAX = sched["smax"]
    TMAX = sched["tmax"]
    IDXC = int(sched["idx_cols"])
    I2R = int(sched["i2rows"])

    nc = bacc.Bacc("TRN2", debug=False, num_swdge_queues=4)

    # ---- parameters
    xt = nc.declare_dram_parameter("xt", [NT, F], BT, isOutput=False)
    xoT = nc.declare_dram_parameter("xoT", [F, SLAB], BT, isOutput=False)
    idxp = nc.declare_dram_parameter("idx", [P, IDXC], dt.int16, isOutput=False)
    ind2p = nc.declare_dram_parameter("ind2w", [P, I2R], BT, isOutput=False)
    pindp = nc.declare_dram_parameter("pindw", [P, NW * G], BT, isOutput=False)
    o4p = nc.declare_dram_parameter("o4", [P, 32], BT, isOutput=False)
    identp = nc.declare_dram_parameter("ident", [P, P], BT, isOutput=False)
    identfp = nc.declare_dram_parameter("identf", [P, P], DT, isOutput=False)
    wp = {}
    for b in (0, 1):
        for nm, shp, ty in (("Wl1", [F, HID], BT), ("Wr1", [F, HID], BT),
                            ("b1", [HID], DT),
                            ("Wl2", [HID, HID], BT), ("Wr2", [HID, HID], BT),
                            ("b2", [HID], DT),
                            ("Wlin", [2 * HID, HID], BT), ("blin", [HID], DT)):
            wp[f"b{b}_{nm}"] = nc.declare_dram_parameter(f"b{b}_{nm}", shp, ty, isOutput=False)
    bns2p = nc.declare_dram_parameter("bns2", [P, 2], DT, isOutput=False)
    bnt2p = nc.declare_dram_parameter("bnt2", [P, 2], DT, isOutput=False)
    l1wp = nc.declare_dram_parameter("l1w", [2 * HID, HID], DT, isOutput=False)
    l1bp = nc.declare_dram_parameter("l1b", [HID], DT, isOutput=False)
    l2wp = nc.declare_dram_parameter("l2w", [HID, C], DT, isOutput=False)
    l2bp = nc.declare_dram_parameter("l2b", [C], DT, isOutput=False)

    out = nc.declare_dram_parameter("out", [G, C], DT, isOutput=True)
    if debug_tables:
        dbgA = nc.declare_dram_parameter("dbgA", [NT, F], BT, isOutput=True)
        dbgB = nc.declare_dram_parameter("dbgB", [NT, F], BT, isOutput=True)

    with tile.TileContext(nc) as tc, ExitStack() as ctx:
        sb = ctx.enter_context(tc.tile_pool(name="sb", bufs=1))
        sb_feat = ctx.enter_context(tc.tile_pool(name="sb_feat", bufs=1))
        sb_g = ctx.enter_context(tc.tile_pool(name="sb_g", bufs=6))
        sb_i2 = ctx.enter_context(tc.tile_pool(name="sb_i2", bufs=4))
        sb_bs = ctx.enter_context(tc.tile_pool(name="sb_bs", bufs=4))
        sb_ms = ctx.enter_context(tc.tile_pool(name="sb_ms", bufs=3))
        ps_bs = ctx.enter_context(tc.tile_pool(name="ps_bs", bufs=3, space="PSUM"))
        ps_agg = ctx.enter_context(tc.tile_pool(name="ps_agg", bufs=2, space="PSUM"))
        ps_mm = ctx.enter_context(tc.tile_pool(name="ps_mm", bufs=2, space="PSUM"))
        ps_pool = ctx.enter_context(tc.tile_pool(name="ps_pool", bufs=1, space="PSUM"))
        dram = ctx.enter_context(tc.tile_pool(name="dram", bufs=1, space="DRAM"))

        nc.gpsimd.load_library(library_config.mlp)

        # ---- constants into SBUF
        o4_t = sb.tile([P, 32], BT)
        nc.sync.dma_start(o4_t[:], o4p[:])
        id_t = sb.tile([P, P], BT)
        nc.sync.dma_start(id_t[:], identp[:])
        idf_t = sb.tile([P, P], DT)
        nc.sync.dma_start(idf_t[:], identfp[:])
        ix_t = sb.tile([P, IDXC], dt.int16)
        nc.sync.dma_start(ix_t[:], idxp[:])
        pind_t = sb.tile([P, NW * G], BT)
        nc.sync.dma_start(pind_t[:], pindp[:])
        wt = {}
        for b in (0, 1):
            for nm in ("Wl1", "Wr1", "Wl2", "Wr2"):
                w_t = sb.tile([P, P], BT, name=f"w{b}{nm}")
                nc.sync.dma_start(w_t[:], wp[f"b{b}_{nm}"][:])
                wt[f"b{b}_{nm}"] = w_t
            wlin_t = sb.tile([P, 2, P], BT, name=f"w{b}lin")
            nc.sync.dma_start(wlin_t[:, 0, :], wp[f"b{b}_Wlin"][0:P, :])
            nc.sync.dma_start(wlin_t[:, 1, :], wp[f"b{b}_Wlin"][P:2 * P, :])
            wt[f"b{b}_Wlin"] = wlin_t
            for nm in ("b1", "b2", "blin"):
                b_t = sb.tile([P, 1], DT, name=f"b{b}{nm}")
                nc.sync.dma_start(b_t[:], wp[f"b{b}_{nm}"][:, None])
                wt[f"b{b}_{nm}"] = b_t
        bns_t = sb.tile([P, 2], DT)
        nc.sync.dma_start(bns_t[:], bns2p[:])
        bnt_t = sb.tile([P, 2], DT)
        nc.sync.dma_start(bnt_t[:], bnt2p[:])
        l1w_t = sb.tile([P, 2, P], DT)
        nc.sync.dma_start(l1w_t[:, 0, :], l1wp[0:P, :])
        nc.sync.dma_start(l1w_t[:, 1, :], l1wp[P:2 * P, :])
        l1b_t = sb.tile([P, 1], DT)
        nc.sync.dma_start(l1b_t[:], l1bp[:, None])
        l2w_t = sb.tile([P, C], DT)
        nc.sync.dma_start(l2w_t[:], l2wp[:])
        l2b_t = sb.tile([P, 1], DT)
        nc.sync.dma_start(l2b_t[0:C, :], l2bp[:, None])

        # feature-major activation buffers [128, SLAB]
        featA = sb_feat.tile([P, SLAB], BT)   # x_ownT, later h (block0 out), h' ...
        featB = sb_feat.tile([P, SLAB], BT)   # h1, h1'
        featC = sb_feat.tile([P, SLAB], BT)   # h2, h2'
        nc.sync.dma_start(featA[:], xoT[:])

        zero_t = sb.tile([P, P], BT)
        nc.vector.memset(zero_t[:], 0.0)

        # DRAM scratch
        cA = dram.tile([SLAB, F], BT)
        cB = dram.tile([SLAB, F], BT)
        tabA = dram.tile([NT, F], BT, addr_space="Shared")
        tabB = dram.tile([NT, F], BT, addr_space="Shared")
        tabC = dram.tile([NT, F], BT, addr_space="Shared")
        pc_in = dram.tile([P, 2 * G], DT)
        pc_out = dram.tile([P, 2 * G], DT, addr_space="Shared")
        nc.sync.dma_start(cA[NPC:SLAB, :], zero_t[0:SLAB - NPC, :])
        nc.sync.dma_start(cB[NPC:SLAB, :], zero_t[0:SLAB - NPC, :])

        qctr = [0]
        ACT_COPY = mybir.ActivationFunctionType.Copy

        def jk_window(w, h1, h2, hout, Wlin, bcol, contrib, pool_ps):
            h_ps = ps_mm.tile([P, P], dt.float32, name="jk_ps", tag="mm")
            nc.tensor.matmul(h_ps[:], Wlin[:, 0, :], h1[:, w * P:(w + 1) * P], start=True, stop=False)
            nc.tensor.matmul(h_ps[:], Wlin[:, 1, :], h2[:, w * P:(w + 1) * P], start=False, stop=True)
            nc.vector.tensor_scalar(hout[:, w * P:(w + 1) * P], h_ps[:], bcol[:], 0.0, Alu.add, Alu.max)
            hnm_ps = ps_mm.tile([P, P], BT, name="jknm_ps", tag="mm")
            nc.tensor.transpose(hnm_ps[:], hout[:, w * P:(w + 1) * P], id_t[:])
            hnm_sb = sb_ms.tile([P, P], BT, name="jknm_sb")
            nc.vector.tensor_copy(hnm_sb[:], hnm_ps[:])
            if contrib is not None:
                rows = min(P, NPC - w * P)
                nc.scalar.dma_start(contrib[w * P:w * P + rows, :], hnm_sb[0:rows, :])
            nc.tensor.matmul(pool_ps[:], hnm_sb[:], pind_t[:, w * G:(w + 1) * G],
                             start=(w == 0), stop=(w == NW - 1))

        def conv(tab, in_feat, out_feat, Wl, Wr, bcol, contrib, jk_spec=None):
            """One SAGE conv: out_feat[:, n] = relu(mean@Wl + in@Wr + b).
            If contrib is not None also write node-major rows there.
            If jk_spec is given, the JumpingKnowledge/pool window of the
            previous conv pair is interleaved after each window."""
            if not hasattr(conv, "first"):
                conv.first = True
            if jk_spec is not None:
                pool_ps = ps_pool.tile([P, G], dt.float32, name="pool_ps")
            for w in range(NW):
                nLw, nHw, Sw, Tw = int(nL[w]), int(nH[w]), int(S[w]), int(T[w])
                g_t = sb_g.tile([P, SMAX // P, P], BT, name="g_t")
                if conv.first and w < 6:
                    # gathers skip per-core trailing slack (-1 idx); zero the
                    # buffers once so skipped slots never hold NaN patterns
                    nc.vector.memset(g_t[:], 0.0)
                for nsec, col0, slot0, view in (
                        (nLw, int(colL[w]), 0, tab[0:LO]),
                        (nHw, int(colH[w]), nLw // P, tab[LO:NT])):
                    nc.gpsimd.dma_gather(
                        g_t[:, slot0:slot0 + nsec // P, :], view,
                        ix_t[:, col0:col0 + nsec // 16],
                        nsec, nsec, P, single_packet=False,
                        queue_num=qctr[0] % 4)
                    qctr[0] += 1

                i2 = sb_i2.tile([P, TMAX * P], BT, name="i2")
                r0 = int(i2off[w])
                nc.sync.dma_start(i2[:, 0:Tw * P], ind2p[:, r0:r0 + Tw * P])

                agg = ps_agg.tile([P, P], dt.float32, name="agg")
                ngrp = Sw // P
                for t in range(Tw):
                    jn = min(4, ngrp - t * 4)
                    bs_ps = ps_bs.tile([P, P], dt.float32, name="bs_ps")
                    for jj in range(jn):
                        j = t * 4 + jj
                        nc.tensor.matmul(
                            bs_ps[32 * jj:32 * jj + 32, :], o4_t[:], g_t[:, j, :],
                            start=True, stop=True, tile_position=(0, 32 * jj))
                    Kt = jn * 32
                    bs_sb = sb_bs.tile([P, P], BT, name="bs_sb")
                    nc.scalar.activation(bs_sb[0:Kt, :], bs_ps[0:Kt, :], ACT_COPY)
                    nc.tensor.matmul(agg[:], bs_sb[0:Kt, :], i2[0:Kt, t * P:t * P + P],
                                     start=(t == 0), stop=(t == Tw - 1))

                mT_sb = sb_ms.tile([P, P], BT, name="mT_sb")
                nc.vector.tensor_copy(mT_sb[:], agg[:])
                h_ps = ps_mm.tile([P, P], dt.float32, name="h_ps", tag="mm")
                nc.tensor.matmul(h_ps[:], Wl[:], mT_sb[:], start=True, stop=False)
                nc.tensor.matmul(h_ps[:], Wr[:], in_feat[:, w * P:(w + 1) * P], start=False, stop=True)
                nc.vector.tensor_scalar(out_feat[:, w * P:(w + 1) * P], h_ps[:], bcol[:], 0.0, Alu.add, Alu.max)
                if contrib is not None:
                    rows = min(P, NPC - w * P)
                    hnm_ps = ps_mm.tile([P, P], BT, name="hnm_ps", tag="mm")
                    nc.tensor.transpose(hnm_ps[:], out_feat[:, w * P:(w + 1) * P], id_t[:])
                    hnm_sb = sb_ms.tile([P, P], BT, name="hnm_sb")
                    nc.scalar.activation(hnm_sb[:], hnm_ps[:], ACT_COPY)
                    nc.scalar.dma_start(contrib[w * P:w * P + rows, :], hnm_sb[0:rows, :])
                if jk_spec is not None:
                    jk_window(w, *jk_spec[:6], pool_ps)
            if jk_spec is not None:
                nc.vector.tensor_copy(jk_spec[6][:], pool_ps[:])
            conv.first = False

        def allgather(contrib, tab):
            nc.gpsimd.collective_compute(
                "AllGather", Alu.bypass, ins=[contrib[:]], outs=[tab[:]],
                replica_groups=[list(range(NCORES))])

        # ---------------- block 0
        conv(xt, featA, featB, wt["b0_Wl1"], wt["b0_Wr1"], wt["b0_b1"], cA)   # h1
        allgather(cA, tabA)
        if n_convs >= 2:
            p0_sb = sb.tile([P, G], DT)
            # conv2 (h2) with JK/pool of block0 interleaved per window
            conv(tabA, featB, featC, wt["b0_Wl2"], wt["b0_Wr2"], wt["b0_b2"], None,
                 jk_spec=(featB, featC, featA, wt["b0_Wlin"], wt["b0_blin"], cB, p0_sb))
            allgather(cB, tabB)
        if n_convs >= 3:
            conv(tabB, featA, featB, wt["b1_Wl1"], wt["b1_Wr1"], wt["b1_b1"], cA)  # h1'
            allgather(cA, tabC)
        if n_convs >= 4:
            p1_sb = sb.tile([P, G], DT)
            conv(tabC, featB, featC, wt["b1_Wl2"], wt["b1_Wr2"], wt["b1_b2"], None,
                 jk_spec=(featB, featC, featA, wt["b1_Wlin"], wt["b1_blin"], None, p1_sb))

            # ---------------- pooling allreduce + head
            nc.sync.dma_start(pc_in[:, 0:G], p0_sb[:])
            nc.sync.dma_start(pc_in[:, G:2 * G], p1_sb[:])
            nc.gpsimd.collective_compute(
                "AllReduce", Alu.add, ins=[pc_in[:]], outs=[pc_out[:]],
                replica_groups=[list(range(NCORES))])
            pools_sb = sb.tile([P, 2 * G], DT)
            nc.sync.dma_start(pools_sb[:], pc_out[:])

            # BN (folded) per feature chunk
            gbn = sb.tile([P, 2, G], DT)
            for k in range(2):
                nc.vector.tensor_scalar(gbn[:, k, :], pools_sb[:, k * G:(k + 1) * G],
                                        bns_t[:, k:k + 1], bnt_t[:, k:k + 1],
                                        Alu.mult, Alu.add)
            l1_ps = ps_mm.tile([P, G], dt.float32, name="l1_ps", tag="mm")
            for k in range(2):
                nc.tensor.matmul(l1_ps[:], l1w_t[:, k, :], gbn[:, k, :],
                                 start=(k == 0), stop=(k == 1))
            z1 = sb.tile([P, G], DT)
            nc.vector.tensor_scalar(z1[:], l1_ps[:], l1b_t[:], 0.0, Alu.add, Alu.max)
            l2_ps = ps_mm.tile([P, G], dt.float32, name="l2_ps", tag="mm")
            nc.tensor.matmul(l2_ps[0:C, :], l2w_t[:], z1[:], start=True, stop=True)
            z2 = sb.tile([P, G], DT)
            nc.vector.tensor_scalar(z2[0:C, :], l2_ps[0:C, :], l2b_t[0:C, :], None, Alu.add)

            # softmax over C (partition dim) -> transpose to [G, C] first
            for half in range(2):
                zt_ps = ps_mm.tile([P, C], dt.float32, name="zt_ps", tag="mm")
                nc.tensor.transpose(zt_ps[:, 0:C], z2[0:C, half * P:(half + 1) * P], idf_t[0:C, 0:C])
                znm = sb.tile([P, C], DT, name=f"znm{half}")
                nc.vector.tensor_copy(znm[:], zt_ps[:, 0:C])
                nmax = sb.tile([P, 1], DT, name=f"nmax{half}")
                nc.vector.tensor_reduce(nmax[:], znm[:], mybir.AxisListType.X, Alu.max, negate=True)
                e_t = sb.tile([P, C], DT, name=f"e_t{half}")
                nc.scalar.activation(e_t[:], znm[:], mybir.ActivationFunctionType.Exp,
                                     bias=nmax[:], scale=1.0)
                ssum = sb.tile([P, 1], DT, name=f"ssum{half}")
                nc.vector.tensor_reduce(ssum[:], e_t[:], mybir.AxisListType.X, Alu.add)
                rcp = sb.tile([P, 1], DT, name=f"rcp{half}")
                nc.vector.reciprocal(rcp[:], ssum[:])
                sm = sb.tile([P, C], DT, name=f"sm{half}")
                nc.vector.tensor_scalar(sm[:], e_t[:], rcp[:], None, Alu.mult)
                nc.sync.dma_start(out[half * P:(half + 1) * P, :], sm[:])

        if debug_tables:
            nc.sync.dma_start(dbgA[:], tabA[:])
            if n_convs >= 2:
                nc.sync.dma_start(dbgB[:], tabB[:])

    nc.compile()
    return nc


# ------------------------------------------------------------------ runtime
def _install_profile_hook():
    try:
        from trn_agent_boot.trn_boot import _ntff_profile_via_ctypes
        hook = _ntff_profile_via_ctypes("/opt/axon/libaxon_pjrt.so")
        m = types.ModuleType("antenv.axon_hooks")
        m.get_axon_ntff_profile_hook = lambda: hook
        sys.modules.setdefault("antenv.axon_hooks", m)
    except Exception:
        pass


def kernel(**inputs):
    from concourse.bass_utils import run_bass_kernel_spmd

    n_convs = int(os.environ.get("KNC_CONVS", "4"))
    debug_tables = bool(int(os.environ.get("KDBG", "0")))
    trace = bool(int(os.environ.get("KTRACE", "0")))
    if trace:
        _install_profile_hook()

    sched, in_maps = _host_inputs(inputs)

    key = (n_convs, debug_tables,
           tuple(int(v) for v in sched["S"][:8]), int(sched["i2rows"]))
    nc = _prog_cache.get(key)
    if nc is None:
        nc = _build_program(sched, n_convs=n_convs, debug_tables=debug_tables)
        _prog_cache[key] = nc

    res = run_bass_kernel_spmd(nc, in_maps, list(range(NCORES)), trace=trace)
    kernel.last_result = res
    out = res.results[0]["out"].astype(np.float32)
    return out


# revision 28
# speedup vs baseline: 1.0282x; 1.0041x over previous
"""GraphSAGE 2-block GNN (nn_BaselineModel_80607946211554) on 8 TRN2 NeuronCores.

Strategy: destination-node sharding. Each core owns a contiguous range of
6250 nodes. Node-feature tables (x, and intermediate h tables) are replicated
in each core's DRAM in a "slab" layout: node n -> table row (n//6250)*6272 +
n%6250, with 22 zero pad rows per slab. Neighbor aggregation is done by
dma_gather of source rows (edges sorted by destination, host-preprocessed)
followed by a fixed ones-block matmul (sums blocks of 4 slots, col-tiled on
the PE array) and a per-window indicator matmul mapping blocks to
destinations. SAGE linear layers run feature-major (weights stationary on the
PE). Intermediate node tables are rebuilt across cores with AllGather
collectives. Graph pooling is a one-hot matmul; the tiny MLP head + softmax is
computed redundantly on every core.

v2: bf16 end-to-end (tables, gathers, matmuls; fp32 PSUM accumulation and
fp32 head), whole-conv gather-index preload, pre-wrapped per-window indicator
and pooling tables (single DMA issue each), full-section gather calls.

Self-contained: hardcodes all shapes for the fixed problem instance.
"""
import os
import sys
import types
import numpy as np
import ml_dtypes

BF = ml_dtypes.bfloat16

N = 50000
E = 1600000
G = 256
F = 128
HID = 128
C = 10
NCORES = 8
NPC = N // NCORES            # 6250 nodes per core
SLAB = 6272                  # slab rows (6250 + 22 zero pad)
NT = NCORES * SLAB           # 50176 table rows
LO = 4 * SLAB                # 25088 (legacy; unused in split-table layout)
SPLIT = 3200                 # slab rows < SPLIT go to table1, rest to table2
NPC2 = NPC - SPLIT           # 3050 rows per core in table2
C1 = SPLIT + 2               # 3202: per-core table1 chunk (2 zero pad rows)
C2 = NPC2 + 2                # 3052: per-core table2 chunk
T1R = NCORES * C1            # 25616 rows
T2R = NCORES * C2            # 24416 rows
PAD1 = SPLIT                 # 3200: core-0 zero row in table1
PAD2 = NPC2                  # 3050: core-0 zero row in table2
CR = C1 + C2                 # 6254: contrib buffer rows
P = 128
NW = (NPC + P - 1) // P      # 49 dst windows per core
EPS = 1e-5

_prog_cache = {}


# ----------------------------------------------------------------- host prep
def _wrap_idx(sec):
    """int64 slot values (len mult of 16) -> [128, n/16] int16 wrapped layout."""
    n = len(sec)
    arr = sec.reshape(n // 16, 16).T.astype(np.int16)   # [16, n/16]
    return np.tile(arr, (8, 1))                          # [128, n/16]


def _build_schedule(src, dst, invd_full):
    """Shared static schedule + per-core gather/indicator data.
    ind2 entries carry 1/deg(dst) so stage-2 emits the neighbourhood mean."""
    core_edges = []
    SL = np.zeros((NCORES, NW), np.int64)
    SH = np.zeros((NCORES, NW), np.int64)
    for c in range(NCORES):
        m = (dst >= c * NPC) & (dst < (c + 1) * NPC)
        s = src[m].astype(np.int64)
        d = (dst[m] - c * NPC).astype(np.int64)
        hi = ((s % NPC) >= SPLIT).astype(np.int64)
        w = d >> 7
        order = np.lexsort((d, hi, w))
        s, d, hi, w = s[order], d[order], hi[order], w[order]
        core_edges.append((s, d, hi, w))
        cnt = np.bincount(d * 2 + hi, minlength=NPC * 2).reshape(NPC, 2)
        pl = ((cnt + 3) >> 2) << 2
        plp = np.zeros((NW * P, 2), np.int64)
        plp[:NPC] = pl
        plw = plp.reshape(NW, P, 2).sum(1)
        SL[c], SH[c] = plw[:, 0], plw[:, 1]

    nL = np.maximum(((SL.max(0) + 127) // P) * P, P).astype(np.int64)
    nH = np.maximum(((SH.max(0) + 127) // P) * P, P).astype(np.int64)
    S = nL + nH
    B = S // 4
    T = (B + 127) // P
    colL = np.zeros(NW, np.int64)
    colH = np.zeros(NW, np.int64)
    off = 0
    for w in range(NW):
        colL[w] = off
        off += nL[w] // 16
        colH[w] = off
        off += nH[w] // 16
    idx_cols = off
    i2off = np.zeros(NW, np.int64)
    o = 0
    for w in range(NW):
        i2off[w] = o
        o += T[w] * P
    i2rows = o

    sched = dict(nL=nL, nH=nH, S=S, B=B, T=T, colL=colL, colH=colH,
                 idx_cols=idx_cols, i2off=i2off, i2rows=i2rows,
                 smax=int(S.max()), tmax=int(T.max()))

    per_core = []
    for c in range(NCORES):
        s, d, hi, w = core_edges[c]
        cnt = np.bincount(d * 2 + hi, minlength=NPC * 2).reshape(NPC, 2)
        pl = ((cnt + 3) >> 2) << 2
        plp = np.zeros((NW * P, 2), np.int64)
        plp[:NPC] = pl
        plw3 = plp.reshape(NW, P, 2)
        gstart = np.cumsum(plw3, axis=1) - plw3           # [NW,128,2]
        key = d * 2 + hi
        if len(key):
            grp_change = np.r_[True, key[1:] != key[:-1]]
            gidx = np.cumsum(grp_change) - 1
            first_pos = np.flatnonzero(grp_change)
            rank = np.arange(len(d)) - first_pos[gidx]
        else:
            rank = np.zeros(0, np.int64)
        pos = gstart[w, d & 127, hi] + rank
        sc, sr = s // NPC, s % NPC
        val = np.where(hi == 1, sc * C2 + (sr - SPLIT), sc * C1 + sr)

        idx_arr = np.zeros((P, idx_cols), np.int16)
        ind2 = np.zeros((i2rows, P), np.float32)
        for wi in range(NW):
            mw = w == wi
            mL = mw & (hi == 0)
            mH = mw & (hi == 1)
            d0, d1w = wi * P, min((wi + 1) * P, NPC)
            own_l = int(pl[d0:d1w, 0].sum())
            own_h = int(pl[d0:d1w, 1].sum())
            # tail slots beyond this core's own (group-padded) usage are -1:
            # the gather ucode truncates trailing negatives before descriptor
            # generation, so they cost no SWDGE emission time. The stale g_t
            # data they leave is nullified by this core's zero ind2 entries.
            secL = np.full(nL[wi], PAD1, np.int64)
            secH = np.full(nH[wi], PAD2, np.int64)
            secL[pos[mL]] = val[mL]
            secH[pos[mH]] = val[mH]
            idx_arr[:, colL[wi]:colL[wi] + nL[wi] // 16] = _wrap_idx(secL)
            idx_arr[:, colH[wi]:colH[wi] + nH[wi] // 16] = _wrap_idx(secH)
            d0, d1 = wi * P, min((wi + 1) * P, NPC)
            dloc = np.arange(d1 - d0)
            bL = np.repeat(dloc, pl[d0:d1, 0] // 4)
            bH = np.repeat(dloc, pl[d0:d1, 1] // 4)
            b2d = np.full(T[wi] * P, -1, np.int64)
            b2d[:len(bL)] = bL
            b2d[nL[wi] // 4:nL[wi] // 4 + len(bH)] = bH
            rows = np.arange(T[wi] * P)
            vmask = b2d >= 0
            blk = ind2[i2off[wi]:i2off[wi] + T[wi] * P]
            blk[rows[vmask], b2d[vmask]] = invd_full[c * NPC + wi * P + b2d[vmask]]
        # pre-wrap indicator to partition-major: ind2w[p, j*128+q] = ind2[j*128+p, q]
        ind2w = (ind2.reshape(i2rows // P, P, P)
                 .transpose(1, 0, 2).reshape(P, i2rows).astype(BF))
        per_core.append(dict(idx=idx_arr, ind2w=ind2w))
    return sched, per_core


def _host_inputs(inputs):
    f32 = lambda a: np.asarray(a, np.float32)
    bf16 = lambda a: np.asarray(a, np.float32).astype(BF)
    x = f32(inputs["x"])
    ei = np.asarray(inputs["edge_index"], np.int64)
    batch = np.asarray(inputs["batch"], np.int64)
    src, dst = ei[0], ei[1]

    deg = np.bincount(dst, minlength=N).astype(np.float32)
    invd_full = (1.0 / np.maximum(deg, 1.0)).astype(np.float32)

    sched, per_core = _build_schedule(src, dst, invd_full)

    xt1 = np.zeros((T1R, F), BF)
    xt2 = np.zeros((T2R, F), BF)
    for r in range(NCORES):
        xt1[r * C1:r * C1 + SPLIT] = x[r * NPC:r * NPC + SPLIT].astype(BF)
        xt2[r * C2:r * C2 + NPC2] = x[r * NPC + SPLIT:(r + 1) * NPC].astype(BF)

    o4 = np.zeros((P, 32), np.float32)
    for e in range(P):
        o4[e, e // 4] = 1.0
    ident = np.eye(P, dtype=np.float32)

    # BN folding
    s_bn = f32(inputs["bn_gamma"]) / np.sqrt(f32(inputs["bn_rv"]) + EPS)
    t_bn = f32(inputs["bn_beta"]) - f32(inputs["bn_rm"]) * s_bn
    bns2 = s_bn.reshape(2, P).T.copy()     # [128, 2]
    bnt2 = t_bn.reshape(2, P).T.copy()

    shared = {
        "xt1": xt1, "xt2": xt2, "o4": o4.astype(BF), "ident": ident.astype(BF),
        "identf": ident, "bns2": bns2, "bnt2": bnt2,
        "l1w": f32(inputs["lin1_W"]), "l1b": f32(inputs["lin1_b"]),
        "l2w": f32(inputs["lin2_W"]), "l2b": f32(inputs["lin2_b"]),
    }
    for b in (0, 1):
        for nm in ("Wl1", "Wr1", "Wl2", "Wr2", "Wlin"):
            shared[f"b{b}_{nm}"] = bf16(inputs[f"b{b}_{nm}"])
        for nm in ("b1", "b2", "blin"):
            shared[f"b{b}_{nm}"] = f32(inputs[f"b{b}_{nm}"])

    in_maps = []
    for c in range(NCORES):
        xoT = np.zeros((F, SLAB), BF)
        xoT[:, :NPC] = x[c * NPC:(c + 1) * NPC].T.astype(BF)
        pool_ind = np.zeros((NW, P, G), np.float32)
        bt = batch[c * NPC:(c + 1) * NPC]
        btp = np.full(NW * P, -1, np.int64)
        btp[:NPC] = bt
        btp2 = btp.reshape(NW, P)
        for wi in range(NW):
            vm = btp2[wi] >= 0
            pool_ind[wi, np.arange(P)[vm], btp2[wi][vm]] = 1.0
        # pre-wrap pooling indicator: pindw[p, w*G+g] = pool_ind[w, p, g]
        pindw = pool_ind.transpose(1, 0, 2).reshape(P, NW * G).astype(BF)
        im = dict(shared)
        im.update({
            "xoT": xoT, "pindw": pindw,
            "idx": per_core[c]["idx"], "ind2w": per_core[c]["ind2w"],
        })
        in_maps.append(im)
    return sched, in_maps


# ------------------------------------------------------------- bass program
def _build_program(sched, n_convs=4, debug_tables=False):
    import concourse.bass as bass
    import concourse.mybir as mybir
    import concourse.tile as tile
    from concourse import bacc
    from concourse import library_config
    from contextlib import ExitStack

    dt = mybir.dt
    DT = dt.float32
    BT = dt.bfloat16
    Alu = mybir.AluOpType

    nL, nH, S, B, T = (sched[k] for k in ("nL", "nH", "S", "B", "T"))
    colL, colH, i2off = sched["colL"], sched["colH"], sched["i2off"]
    SMAX = sched["smax"]
    TMAX = sched["tmax"]
    IDXC = int(sched["idx_cols"])
    I2R = int(sched["i2rows"])

    nc = bacc.Bacc("TRN2", debug=False, num_swdge_queues=4)

    # ---- parameters
    xt1p = nc.declare_dram_parameter("xt1", [T1R, F], BT, isOutput=False)
    xt2p = nc.declare_dram_parameter("xt2", [T2R, F], BT, isOutput=False)
    xoT = nc.declare_dram_parameter("xoT", [F, SLAB], BT, isOutput=False)
    idxp = nc.declare_dram_parameter("idx", [P, IDXC], dt.int16, isOutput=False)
    ind2p = nc.declare_dram_parameter("ind2w", [P, I2R], BT, isOutput=False)
    pindp = nc.declare_dram_parameter("pindw", [P, NW * G], BT, isOutput=False)
    o4p = nc.declare_dram_parameter("o4", [P, 32], BT, isOutput=False)
    identp = nc.declare_dram_parameter("ident", [P, P], BT, isOutput=False)
    identfp = nc.declare_dram_parameter("identf", [P, P], DT, isOutput=False)
    wp = {}
    for b in (0, 1):
        for nm, shp, ty in (("Wl1", [F, HID], BT), ("Wr1", [F, HID], BT),
                            ("b1", [HID], DT),
                            ("Wl2", [HID, HID], BT), ("Wr2", [HID, HID], BT),
                            ("b2", [HID], DT),
                            ("Wlin", [2 * HID, HID], BT), ("blin", [HID], DT)):
            wp[f"b{b}_{nm}"] = nc.declare_dram_parameter(f"b{b}_{nm}", shp, ty, isOutput=False)
    bns2p = nc.declare_dram_parameter("bns2", [P, 2], DT, isOutput=False)
    bnt2p = nc.declare_dram_parameter("bnt2", [P, 2], DT, isOutput=False)
    l1wp = nc.declare_dram_parameter("l1w", [2 * HID, HID], DT, isOutput=False)
    l1bp = nc.declare_dram_parameter("l1b", [HID], DT, isOutput=False)
    l2wp = nc.declare_dram_parameter("l2w", [HID, C], DT, isOutput=False)
    l2bp = nc.declare_dram_parameter("l2b", [C], DT, isOutput=False)

    out = nc.declare_dram_parameter("out", [G, C], DT, isOutput=True)
    if debug_tables:
        dbgA = nc.declare_dram_parameter("dbgA", [NT, F], BT, isOutput=True)
        dbgB = nc.declare_dram_parameter("dbgB", [NT, F], BT, isOutput=True)

    with tile.TileContext(nc) as tc, ExitStack() as ctx:
        sb = ctx.enter_context(tc.tile_pool(name="sb", bufs=1))
        sb_feat = ctx.enter_context(tc.tile_pool(name="sb_feat", bufs=1))
        sb_g = ctx.enter_context(tc.tile_pool(name="sb_g", bufs=6))
        sb_i2 = ctx.enter_context(tc.tile_pool(name="sb_i2", bufs=4))
        sb_bs = ctx.enter_context(tc.tile_pool(name="sb_bs", bufs=4))
        sb_ms = ctx.enter_context(tc.tile_pool(name="sb_ms", bufs=3))
        ps_bs = ctx.enter_context(tc.tile_pool(name="ps_bs", bufs=3, space="PSUM"))
        ps_agg = ctx.enter_context(tc.tile_pool(name="ps_agg", bufs=2, space="PSUM"))
        ps_mm = ctx.enter_context(tc.tile_pool(name="ps_mm", bufs=2, space="PSUM"))
        ps_pool = ctx.enter_context(tc.tile_pool(name="ps_pool", bufs=1, space="PSUM"))
        dram = ctx.enter_context(tc.tile_pool(name="dram", bufs=1, space="DRAM"))

        nc.gpsimd.load_library(library_config.mlp)

        # ---- constants into SBUF
        o4_t = sb.tile([P, 32], BT)
        nc.sync.dma_start(o4_t[:], o4p[:])
        id_t = sb.tile([P, P], BT)
        nc.sync.dma_start(id_t[:], identp[:])
        idf_t = sb.tile([P, P], DT)
        nc.sync.dma_start(idf_t[:], identfp[:])
        ix_t = sb.tile([P, IDXC], dt.int16)
        nc.sync.dma_start(ix_t[:], idxp[:])
        pind_t = sb.tile([P, NW * G], BT)
        nc.sync.dma_start(pind_t[:], pindp[:])
        wt = {}
        for b in (0, 1):
            for nm in ("Wl1", "Wr1", "Wl2", "Wr2"):
                w_t = sb.tile([P, P], BT, name=f"w{b}{nm}")
                nc.sync.dma_start(w_t[:], wp[f"b{b}_{nm}"][:])
                wt[f"b{b}_{nm}"] = w_t
            wlin_t = sb.tile([P, 2, P], BT, name=f"w{b}lin")
            nc.sync.dma_start(wlin_t[:, 0, :], wp[f"b{b}_Wlin"][0:P, :])
            nc.sync.dma_start(wlin_t[:, 1, :], wp[f"b{b}_Wlin"][P:2 * P, :])
            wt[f"b{b}_Wlin"] = wlin_t
            for nm in ("b1", "b2", "blin"):
                b_t = sb.tile([P, 1], DT, name=f"b{b}{nm}")
                nc.sync.dma_start(b_t[:], wp[f"b{b}_{nm}"][:, None])
                wt[f"b{b}_{nm}"] = b_t
        bns_t = sb.tile([P, 2], DT)
        nc.sync.dma_start(bns_t[:], bns2p[:])
        bnt_t = sb.tile([P, 2], DT)
        nc.sync.dma_start(bnt_t[:], bnt2p[:])
        l1w_t = sb.tile([P, 2, P], DT)
        nc.sync.dma_start(l1w_t[:, 0, :], l1wp[0:P, :])
        nc.sync.dma_start(l1w_t[:, 1, :], l1wp[P:2 * P, :])
        l1b_t = sb.tile([P, 1], DT)
        nc.sync.dma_start(l1b_t[:], l1bp[:, None])
        l2w_t = sb.tile([P, C], DT)
        nc.sync.dma_start(l2w_t[:], l2wp[:])
        l2b_t = sb.tile([P, 1], DT)
        nc.sync.dma_start(l2b_t[0:C, :], l2bp[:, None])

        # feature-major activation buffers [128, SLAB]
        featA = sb_feat.tile([P, SLAB], BT)   # x_ownT, later h (block0 out), h' ...
        featB = sb_feat.tile([P, SLAB], BT)   # h1, h1'
        featC = sb_feat.tile([P, SLAB], BT)   # h2, h2'
        nc.sync.dma_start(featA[:], xoT[:])

        zero_t = sb.tile([P, P], BT)
        nc.vector.memset(zero_t[:], 0.0)

        # DRAM scratch: contrib buffers + three split table pairs
        cA = dram.tile([CR, F], BT)
        cB = dram.tile([CR, F], BT)
        t1A = dram.tile([T1R, F], BT, addr_space="Shared")
        t2A = dram.tile([T2R, F], BT, addr_space="Shared")
        t1B = dram.tile([T1R, F], BT, addr_space="Shared")
        t2B = dram.tile([T2R, F], BT, addr_space="Shared")
        t1C = dram.tile([T1R, F], BT, addr_space="Shared")
        t2C = dram.tile([T2R, F], BT, addr_space="Shared")
        pc_in = dram.tile([P, 2 * G], DT)
        pc_out = dram.tile([P, 2 * G], DT, addr_space="Shared")
        for cbuf in (cA, cB):
            nc.sync.dma_start(cbuf[SPLIT:C1, :], zero_t[0:2, :])
            nc.sync.dma_start(cbuf[C1 + NPC2:CR, :], zero_t[0:2, :])

        qctr = [0]
        ACT_COPY = mybir.ActivationFunctionType.Copy

        def jk_window(w, h1, h2, hout, Wlin, bcol, contrib, pool_ps):
            h_ps = ps_mm.tile([P, P], dt.float32, name="jk_ps", tag="mm")
            nc.tensor.matmul(h_ps[:], Wlin[:, 0, :], h1[:, w * P:(w + 1) * P], start=True, stop=False)
            nc.tensor.matmul(h_ps[:], Wlin[:, 1, :], h2[:, w * P:(w + 1) * P], start=False, stop=True)
            nc.vector.tensor_scalar(hout[:, w * P:(w + 1) * P], h_ps[:], bcol[:], 0.0, Alu.add, Alu.max)
            hnm_ps = ps_mm.tile([P, P], BT, name="jknm_ps", tag="mm")
            nc.tensor.transpose(hnm_ps[:], hout[:, w * P:(w + 1) * P], id_t[:])
            hnm_sb = sb_ms.tile([P, P], BT, name="jknm_sb")
            nc.vector.tensor_copy(hnm_sb[:], hnm_ps[:])
            if contrib is not None:
                rows = min(P, NPC - w * P)
                roff = w * P if w * P < SPLIT else w * P + 2
                nc.scalar.dma_start(contrib[roff:roff + rows, :], hnm_sb[0:rows, :])
            nc.tensor.matmul(pool_ps[:], hnm_sb[:], pind_t[:, w * G:(w + 1) * G],
                             start=(w == 0), stop=(w == NW - 1))

        def conv(tabs, in_feat, out_feat, Wl, Wr, bcol, contrib, jk_spec=None, ag=None):
            """One SAGE conv: out_feat[:, n] = relu(mean@Wl + in@Wr + b).
            If contrib is not None also write node-major rows there.
            If jk_spec is given, the JumpingKnowledge/pool window of the
            previous conv pair is interleaved after each window."""
            if not hasattr(conv, "first"):
                conv.first = True
            if jk_spec is not None:
                pool_ps = ps_pool.tile([P, G], dt.float32, name="pool_ps")
            for w in range(NW):
                nLw, nHw, Sw, Tw = int(nL[w]), int(nH[w]), int(S[w]), int(T[w])
                g_t = sb_g.tile([P, SMAX // P, P], BT, name="g_t")
                if conv.first and w < 6:
                    # gathers skip per-core trailing slack (-1 idx); zero the
                    # buffers once so skipped slots never hold NaN patterns
                    nc.vector.memset(g_t[:], 0.0)
                for nsec, col0, slot0, view in (
                        (nLw, int(colL[w]), 0, tabs[0][:]),
                        (nHw, int(colH[w]), nLw // P, tabs[1][:])):
                    nc.gpsimd.dma_gather(
                        g_t[:, slot0:slot0 + nsec // P, :], view,
                        ix_t[:, col0:col0 + nsec // 16],
                        nsec, nsec, P, single_packet=False,
                        queue_num=qctr[0] % 4)
                    qctr[0] += 1

                i2 = sb_i2.tile([P, TMAX * P], BT, name="i2")
                r0 = int(i2off[w])
                nc.sync.dma_start(i2[:, 0:Tw * P], ind2p[:, r0:r0 + Tw * P])

                agg = ps_agg.tile([P, P], dt.float32, name="agg")
                ngrp = Sw // P
                for t in range(Tw):
                    jn = min(4, ngrp - t * 4)
                    bs_ps = ps_bs.tile([P, P], dt.float32, name="bs_ps")
                    for jj in range(jn):
                        j = t * 4 + jj
                        nc.tensor.matmul(
                            bs_ps[32 * jj:32 * jj + 32, :], o4_t[:], g_t[:, j, :],
                            start=True, stop=True, tile_position=(0, 32 * jj))
                    Kt = jn * 32
                    bs_sb = sb_bs.tile([P, P], BT, name="bs_sb")
                    nc.scalar.activation(bs_sb[0:Kt, :], bs_ps[0:Kt, :], ACT_COPY)
                    nc.tensor.matmul(agg[:], bs_sb[0:Kt, :], i2[0:Kt, t * P:t * P + P],
                                     start=(t == 0), stop=(t == Tw - 1))

                mT_sb = sb_ms.tile([P, P], BT, name="mT_sb")
                nc.vector.tensor_copy(mT_sb[:], agg[:])
                h_ps = ps_mm.tile([P, P], dt.float32, name="h_ps", tag="mm")
                nc.tensor.matmul(h_ps[:], Wl[:], mT_sb[:], start=True, stop=False)
                nc.tensor.matmul(h_ps[:], Wr[:], in_feat[:, w * P:(w + 1) * P], start=False, stop=True)
                nc.vector.tensor_scalar(out_feat[:, w * P:(w + 1) * P], h_ps[:], bcol[:], 0.0, Alu.add, Alu.max)
                if contrib is not None:
                    rows = min(P, NPC - w * P)
                    roff = w * P if w * P < SPLIT else w * P + 2
                    hnm_ps = ps_mm.tile([P, P], BT, name="hnm_ps", tag="mm")
                    nc.tensor.transpose(hnm_ps[:], out_feat[:, w * P:(w + 1) * P], id_t[:])
                    hnm_sb = sb_ms.tile([P, P], BT, name="hnm_sb")
                    nc.scalar.activation(hnm_sb[:], hnm_ps[:], ACT_COPY)
                    nc.scalar.dma_start(contrib[roff:roff + rows, :], hnm_sb[0:rows, :])
                if jk_spec is not None:
                    jk_window(w, *jk_spec[:6], pool_ps)
                if ag is not None and w == SPLIT // P - 1:
                    # first-half contrib rows done: gather them across cores
                    # while the second half still computes
                    nc.gpsimd.collective_compute(
                        "AllGather", Alu.bypass, ins=[ag[0][0:C1]],
                        outs=[ag[1][0:T1R]],
                        replica_groups=[list(range(NCORES))])
            if jk_spec is not None:
                nc.vector.tensor_copy(jk_spec[6][:], pool_ps[:])
            if ag is not None:
                nc.gpsimd.collective_compute(
                    "AllGather", Alu.bypass, ins=[ag[0][C1:CR]],
                    outs=[ag[2][0:T2R]],
                    replica_groups=[list(range(NCORES))])
            conv.first = False

        # ---------------- block 0
        conv((xt1p, xt2p), featA, featB, wt["b0_Wl1"], wt["b0_Wr1"], wt["b0_b1"], cA,
             ag=(cA, t1A, t2A))   # h1
        if n_convs >= 2:
            p0_sb = sb.tile([P, G], DT)
            # conv2 (h2) with JK/pool of block0 interleaved per window
            conv((t1A, t2A), featB, featC, wt["b0_Wl2"], wt["b0_Wr2"], wt["b0_b2"], None,
                 jk_spec=(featB, featC, featA, wt["b0_Wlin"], wt["b0_blin"], cB, p0_sb),
                 ag=(cB, t1B, t2B))
        if n_convs >= 3:
            conv((t1B, t2B), featA, featB, wt["b1_Wl1"], wt["b1_Wr1"], wt["b1_b1"], cA,
                 ag=(cA, t1C, t2C))  # h1'
        if n_convs >= 4:
            p1_sb = sb.tile([P, G], DT)
            conv((t1C, t2C), featB, featC, wt["b1_Wl2"], wt["b1_Wr2"], wt["b1_b2"], None,
                 jk_spec=(featB, featC, featA, wt["b1_Wlin"], wt["b1_blin"], None, p1_sb))

            # ---------------- pooling allreduce + head
            nc.sync.dma_start(pc_in[:, 0:G], p0_sb[:])
            nc.sync.dma_start(pc_in[:, G:2 * G], p1_sb[:])
            nc.gpsimd.collective_compute(
                "AllReduce", Alu.add, ins=[pc_in[:]], outs=[pc_out[:]],
                replica_groups=[list(range(NCORES))])
            pools_sb = sb.tile([P, 2 * G], DT)
            nc.sync.dma_start(pools_sb[:], pc_out[:])

            # BN (folded) per feature chunk
            gbn = sb.tile([P, 2, G], DT)
            for k in range(2):
                nc.vector.tensor_scalar(gbn[:, k, :], pools_sb[:, k * G:(k + 1) * G],
                                        bns_t[:, k:k + 1], bnt_t[:, k:k + 1],
                                        Alu.mult, Alu.add)
            l1_ps = ps_mm.tile([P, G], dt.float32, name="l1_ps", tag="mm")
            for k in range(2):
                nc.tensor.matmul(l1_ps[:], l1w_t[:, k, :], gbn[:, k, :],
                                 start=(k == 0), stop=(k == 1))
            z1 = sb.tile([P, G], DT)
            nc.vector.tensor_scalar(z1[:], l1_ps[:], l1b_t[:], 0.0, Alu.add, Alu.max)
            l2_ps = ps_mm.tile([P, G], dt.float32, name="l2_ps", tag="mm")
            nc.tensor.matmul(l2_ps[0:C, :], l2w_t[:], z1[:], start=True, stop=True)
            z2 = sb.tile([P, G], DT)
            nc.vector.tensor_scalar(z2[0:C, :], l2_ps[0:C, :], l2b_t[0:C, :], None, Alu.add)

            # softmax over C (partition dim) -> transpose to [G, C] first
            for half in range(2):
                zt_ps = ps_mm.tile([P, C], dt.float32, name="zt_ps", tag="mm")
                nc.tensor.transpose(zt_ps[:, 0:C], z2[0:C, half * P:(half + 1) * P], idf_t[0:C, 0:C])
                znm = sb.tile([P, C], DT, name=f"znm{half}")
                nc.vector.tensor_copy(znm[:], zt_ps[:, 0:C])
                nmax = sb.tile([P, 1], DT, name=f"nmax{half}")
                nc.vector.tensor_reduce(nmax[:], znm[:], mybir.AxisListType.X, Alu.max, negate=True)
                e_t = sb.tile([P, C], DT, name=f"e_t{half}")
                nc.scalar.activation(e_t[:], znm[:], mybir.ActivationFunctionType.Exp,
                                     bias=nmax[:], scale=1.0)
                ssum = sb.tile([P, 1], DT, name=f"ssum{half}")
                nc.vector.tensor_reduce(ssum[:], e_t[:], mybir.AxisListType.X, Alu.add)
                rcp = sb.tile([P, 1], DT, name=f"rcp{half}")
                nc.vector.reciprocal(rcp[:], ssum[:])
                sm = sb.tile([P, C], DT, name=f"sm{half}")
                nc.vector.tensor_scalar(sm[:], e_t[:], rcp[:], None, Alu.mult)
                nc.sync.dma_start(out[half * P:(half + 1) * P, :], sm[:])



    nc.compile()
    return nc


# ------------------------------------------------------------------ runtime
def _install_profile_hook():
    try:
        from trn_agent_boot.trn_boot import _ntff_profile_via_ctypes
        hook = _ntff_profile_via_ctypes("/opt/axon/libaxon_pjrt.so")
        m = types.ModuleType("antenv.axon_hooks")
        m.get_axon_ntff_profile_hook = lambda: hook
        sys.modules.setdefault("antenv.axon_hooks", m)
    except Exception:
        pass


def kernel(**inputs):
    from concourse.bass_utils import run_bass_kernel_spmd

    n_convs = int(os.environ.get("KNC_CONVS", "4"))
    debug_tables = bool(int(os.environ.get("KDBG", "0")))
    trace = bool(int(os.environ.get("KTRACE", "0")))
    if trace:
        _install_profile_hook()

    sched, in_maps = _host_inputs(inputs)

    key = (n_convs, debug_tables,
           tuple(int(v) for v in sched["S"][:8]), int(sched["i2rows"]))
    nc = _prog_cache.get(key)
    if nc is None:
        nc = _build_program(sched, n_convs=n_convs, debug_tables=debug_tables)
        _prog_cache[key] = nc

    res = run_bass_kernel_spmd(nc, in_maps, list(range(NCORES)), trace=trace)
    kernel.last_result = res
    out = res.results[0]["out"].astype(np.float32)
    return out
